# revision 4
# baseline (speedup 1.0000x reference)
"""Contrastive-loss Trainium2 kernel v2: symmetry-maximized fp8 GEMM + AllGather.

Over the v1 baseline:
  - DoubleRowSwInterleave perf mode (contiguous weight loads, ~1.6x faster MMs)
  - d=4 block pair computed once globally: each core does two quadrant pieces
    (pieceA = rows[0:512) x d4cols[0:512), pieceB = rows[512:1024) x
    d4cols[512:1024) in the LOCAL frame; the host permutes the d4 column rows
    for cores >= 4 so the four unique d4 blocks are tiled exactly once).
  - d=0 diagonal block: only quadrants Q00 (nb0 x mb0-3), Q01 (nb1 x mb0-3),
    Q11 (nb1 x mb4-7) are computed; rows[512:1024) get Q01's column sums
    locally (skips Q10: 4 psum tiles).
  - pos comes from the d4 piece diagonals: only ranks 0..3 hold real pos
    entries (each serving two global rows); assembly uses 2x their sums.

Per-core GEMM: 68 psum tiles [128,512] x 4 MMs (K=256 DoubleRow) = 272 MMs.

Packet (f32): [rowsum 1024 | cs_d1 1024 | cs_d2 1024 | cs_d3 1024 |
               cs_d4A 512 | cs_d4B 512 | pos 128] = 5248.
Row totals for block b rows: rowsum(rank b) + cs_d{1,2,3}(rank b-d)
  + cs_d4A/B(rank b+4; A covers rows[0:512) iff (b+4)%8 < 4, else swapped).
"""

import os
from contextlib import ExitStack

import numpy as np

N = 8192
D = 1024
N_CORES = 8
ROWS_PER_CORE = N // N_CORES  # 1024
P = 128
TEMPERATURE = 0.07
INV_T = 1.0 / TEMPERATURE
MASK_VAL = -65504.0
SCALE = 16.0  # pre-fp8 scale; psum holds SCALE^2 * cos

NROWS = 1024  # only own rows are loaded; column blocks arrive via AllGather
N_ROW_TILES = NROWS // P  # 8
MB = ROWS_PER_CORE // P  # 8
KT = D // P  # 8
COLG = 512
NB = 10  # column groups: 0,1 own; 2..7 d1..d3; 8,9 d4 pieces
MB_OF_NB = {
    0: range(0, 4),  # Q00
    1: range(0, 8),  # Q01 (mb 0-3) + Q11 (mb 4-7)
    8: range(0, 4),  # d4 piece A
    9: range(4, 8),  # d4 piece B
}
for _nb in range(2, 8):
    MB_OF_NB[_nb] = range(0, 8)

# packet field offsets (floats)
PK_ROWSUM = 0
PK_CS1 = 1024
PK_CS2 = 2048
PK_CS3 = 3072
PK_CS4A = 4096
PK_CS4B = 4608
PK_POS = 5120
PKT = 5248

_CACHE = {}


def make_in_maps(z):
    z = np.ascontiguousarray(np.asarray(z, dtype=np.float32))
    return [
        {"z": np.ascontiguousarray(z[ROWS_PER_CORE * c : ROWS_PER_CORE * (c + 1)])}
        for c in range(N_CORES)
    ]


def _build_nc(repeat=1):
    import concourse.mybir as mybir
    import concourse.tile as tile
    from concourse import bacc
    from concourse.masks import make_identity
    from concourse.bass import _add_dep_helper

    f32 = mybir.dt.float32
    bf16 = mybir.dt.bfloat16
    fp8 = mybir.dt.float8e4
    AF = mybir.ActivationFunctionType
    ALU = mybir.AluOpType

    nc = bacc.Bacc("TRN2")
    z_in = nc.dram_tensor("z", [NROWS, D], f32, kind="ExternalInput")
    out_dram = nc.dram_tensor("out", [P, 1], f32, kind="ExternalOutput")
    pkt_dram = nc.dram_tensor("pkt", [PKT], f32)
    gathered = nc.dram_tensor("gathered", [N_CORES, PKT], f32, addr_space="Shared")
    # fp8 znt exchange: each rank publishes its 2 own groups (1MB)
    zpkt = nc.dram_tensor("zpkt", [2, P, KT // 2, 2 * COLG], fp8)
    gathered_z = nc.dram_tensor(
        "gathered_z", [2 * N_CORES, P, KT // 2, 2 * COLG], fp8, addr_space="Shared"
    )

    ctx = ExitStack()
    with ctx:
        tc = ctx.enter_context(tile.TileContext(nc))
        consts = ctx.enter_context(tc.tile_pool(name="consts", bufs=1))
        znt_pool = ctx.enter_context(tc.tile_pool(name="znt", bufs=1))
        work = ctx.enter_context(tc.tile_pool(name="work", bufs=3))
        zin = ctx.enter_context(tc.tile_pool(name="zin", bufs=4))
        small = ctx.enter_context(tc.tile_pool(name="small", bufs=4))
        accp = ctx.enter_context(tc.tile_pool(name="accp", bufs=1))
        colp = ctx.enter_context(tc.tile_pool(name="colp", bufs=1))
        psum_t = ctx.enter_context(tc.tile_pool(name="psum_t", bufs=2, space="PSUM"))
        psum_mm = ctx.enter_context(tc.tile_pool(name="psum_mm", bufs=3, space="PSUM"))
        psum_cs = ctx.enter_context(tc.tile_pool(name="psum_cs", bufs=1, space="PSUM"))

        ident_f32 = consts.tile([P, P], f32, tag="ident_f32")
        make_identity(nc, ident_f32)
        ident_bf16 = consts.tile([P, P], bf16, tag="ident_bf16")
        make_identity(nc, ident_bf16)
        # anti-identity: permutation that reverses columns in PE transpose
        anti_bf16 = consts.tile([P, P], bf16, tag="anti_bf16")
        nc.gpsimd.memset(anti_bf16, 0.0)
        nc.gpsimd.affine_select(
            out=anti_bf16,
            in_=anti_bf16,
            compare_op=mybir.AluOpType.not_equal,
            fill=1.0,
            base=-(P - 1),
            # on where (x + y - 127) == 0
            pattern=[[1, P]],
            channel_multiplier=1,
        )
        negtile = consts.tile([P, P], f32, tag="negtile")
        nc.vector.memset(negtile, MASK_VAL * SCALE * SCALE)
        ident_u8 = consts.tile([P, P], mybir.dt.uint8, tag="ident_u8")
        nc.vector.tensor_copy(ident_u8, ident_f32)
        ones_col = consts.tile([P, 1], bf16, tag="ones_col")
        nc.vector.memset(ones_col, 1.0)

        # interleaved layout for DoubleRowSwInterleave: znt[g] is [P, 4, 1024]
        # fp8 with flat index (per kk2 group) s = 2*c + i holding
        # znT[k = 128*(2*kk2 + i) + p, 512*g + c]
        znt = [
            znt_pool.tile([P, KT // 2, 2 * COLG], fp8, tag=f"znt{g}", name=f"znt{g}")
            for g in range(NB)
        ]
        # weights copies of the own-row groups (lg 0,1): within each 128-col
        # chunk the columns are REVERSED (via anti-identity transpose), giving
        # the [A_{m+127} B_{m+127} ... A_m B_m] flat order SwInterleave wants
        znt_w = [
            znt_pool.tile([P, KT // 2, 2 * COLG], fp8, tag=f"zntw{g}", name=f"zntw{g}")
            for g in range(2)
        ]

        accs = accp.tile([P, MB, NB], f32, tag="accs")
        posq = accp.tile([P, MB], f32, tag="posq")
        # column-sum accumulators:
        #  colT[0..5] <- nb 2..7 (all mb)     -> cs_d1..cs_d3
        #  colT[6]    <- nb 1, mb 0..3        -> Q01 colsums (local use)
        #  colT[7]    <- nb 8, mb 0..3        -> cs_d4A
        #  colT[8]    <- nb 9, mb 4..7        -> cs_d4B
        colT = [
            colp.tile([P, COLG], f32, tag=f"colT{j}", name=f"colT{j}")
            for j in range(9)
        ]
        cs = colp.tile([P, 36], f32, tag="cs")  # 9 groups x 4 chunks

        for _rep in range(repeat):
            # ---- phase 1: normalize + transpose (local rows [0:5120)) ----
            for t in range(N_ROW_TILES):
                zt = zin.tile([P, 2, D // 2], f32, tag="zt")
                nc.sync.dma_start(
                    out=zt,
                    in_=z_in[t * P : (t + 1) * P, :].rearrange(
                        "p (a b) -> p a b", a=2
                    ),
                )
                stats = small.tile([P, 2, 6], f32, tag="stats")
                nc.vector.bn_stats(out=stats[:, 0, :], in_=zt[:, 0, :])
                nc.vector.bn_stats(out=stats[:, 1, :], in_=zt[:, 1, :])
                mv = small.tile([P, 2], f32, tag="mv")
                nc.vector.bn_aggr(out=mv, in_=stats)
                m2 = small.tile([P, 1], f32, tag="m2")
                nc.vector.tensor_mul(m2, mv[:, 0:1], mv[:, 0:1])
                s2 = small.tile([P, 1], f32, tag="s2")
                nc.vector.tensor_add(s2, m2, mv[:, 1:2])
                nrm = small.tile([P, 1], f32, tag="nrm")
                nc.scalar.activation(nrm, s2, AF.Sqrt, scale=float(D) / (SCALE * SCALE))
                rinv = small.tile([P, 1], f32, tag="rinv")
                nc.vector.reciprocal(rinv, nrm)

                zn_row = work.tile([P, D], bf16, tag="zn_row")
                nc.vector.tensor_scalar_mul(
                    zn_row.rearrange("p (a b) -> p a b", a=2), zt, rinv
                )

                ptr = psum_t.tile([P, KT * P], bf16, tag="ptr")
                for kk in range(KT):
                    nc.tensor.transpose(
                        ptr[:, kk * P : (kk + 1) * P],
                        zn_row[:, kk * P : (kk + 1) * P],
                        ident_bf16,
                    )
                g, col = t // 4, (t % 4) * P
                # interleave into znt: dst[p, kk2, c, i] = ptr[p, (2kk2+i)*128 + (c-col)]
                dstv = znt[g].rearrange("p k (c i) -> p k c i", i=2)
                srcv = ptr.rearrange("p (k2 j c) -> p k2 j c", k2=KT // 2, j=2)
                for i in range(2):
                    dst = dstv[:, :, col : col + P, i]
                    src = srcv[:, :, i, :]
                    if (t + i) % 2 == 0:
                        nc.scalar.copy(dst, src)
                    else:
                        nc.vector.tensor_copy(dst, src)
                if t < 8:
                    # own rows: also build the reversed weights copy
                    ptr2 = psum_t.tile([P, KT * P], bf16, tag="ptr2")
                    for kk in range(KT):
                        nc.tensor.transpose(
                            ptr2[:, kk * P : (kk + 1) * P],
                            zn_row[:, kk * P : (kk + 1) * P],
                            anti_bf16,
                        )
                    dstw = znt_w[g].rearrange("p k (c i) -> p k c i", i=2)
                    srcw = ptr2.rearrange("p (k2 j c) -> p k2 j c", k2=KT // 2, j=2)
                    for i in range(2):
                        dst = dstw[:, :, col : col + P, i]
                        src = srcw[:, :, i, :]
                        if (t + i) % 2 == 0:
                            nc.vector.tensor_copy(dst, src)
                        else:
                            nc.scalar.copy(dst, src)

            # ---- publish own fp8 groups; gather everyone's ----
            import concourse.bass as bass_mod

            zw = [
                nc.sync.dma_start(out=zpkt[g], in_=znt[g]) for g in range(2)
            ]
            ccz = nc.gpsimd.collective_compute(
                "AllGather",
                mybir.AluOpType.bypass,
                ins=[zpkt.ap()],
                outs=[gathered_z.ap()],
                replica_groups=[list(range(N_CORES))],
            )
            for d in zw:
                _add_dep_helper(ccz.ins, d.ins, reason="ccz after zpkt write")
            pid = nc.sync.partition_id()
            # distance-d blocks -> znt[2d], znt[2d+1]; d4 half-blocks by group:
            # pieceA = group (pid//4) of rank (pid+4)%8, pieceB = the other
            for d in (1, 2, 3):
                for g in range(2):
                    idx = ((pid + d) % 8) * 2 + g
                    dz = nc.sync.dma_start(
                        out=znt[2 * d + g],
                        in_=gathered_z[bass_mod.ds(idx, 1)],
                    )
                    _add_dep_helper(dz.ins, ccz.ins, reason="read z after ccz")
            gA = pid // 4
            idxA = ((pid + 4) % 8) * 2 + gA
            idxB = ((pid + 4) % 8) * 2 + (1 - gA)
            for slot, idx in ((8, idxA), (9, idxB)):
                dz = nc.sync.dma_start(
                    out=znt[slot], in_=gathered_z[bass_mod.ds(idx, 1)]
                )
                _add_dep_helper(dz.ins, ccz.ins, reason="read z after ccz")

            for j in range(9):
                nc.vector.memset(colT[j], 0.0)

            # ---- phase 2: GEMM + exp row-sums + colsum accumulation ----
            for nb in range(NB):
                for mb in MB_OF_NB[nb]:
                    ps = psum_mm.tile([P, COLG], f32, tag="ps")
                    lg, lcol = mb // 4, (mb % 4) * P
                    for kk2 in range(KT // 2):
                        w_ap = znt_w[lg][:, kk2, 2 * lcol : 2 * (lcol + P)].rearrange(
                            "p (m i) -> p m i", i=2
                        )
                        x_ap = znt[nb][:, kk2, :].rearrange("p (n i) -> p i n", i=2)
                        nc.tensor.matmul(
                            ps,
                            lhsT=w_ap,
                            rhs=x_ap,
                            perf_mode=mybir.MatmulPerfMode.DoubleRowSwInterleave,
                            start=(kk2 == 0),
                            stop=(kk2 == KT // 2 - 1),
                        )
                    # self-similarity masking on the d0 diagonal tiles
                    if nb == mb // 4:
                        off = (mb % 4) * P
                        nc.vector.copy_predicated(
                            out=ps[:, off : off + P], mask=ident_u8, data=negtile
                        )
                    # pos extraction from the d4 piece diagonals
                    if (nb == 8 and mb < 4) or (nb == 9 and mb >= 4):
                        off = (mb % 4) * P
                        pos_scr = work.tile([P, P], f32, tag="pos_scr")
                        nc.vector.tensor_mul(pos_scr, ps[:, off : off + P], ident_f32)
                        nc.vector.tensor_reduce(
                            posq[:, mb : mb + 1],
                            pos_scr,
                            axis=mybir.AxisListType.X,
                            op=ALU.add,
                        )
                    ex = work.tile([P, COLG], bf16, tag="ex")
                    nc.scalar.activation(
                        ex, ps, AF.Exp, scale=INV_T / (SCALE * SCALE),
                        accum_out=accs[:, mb, nb : nb + 1],
                    )
                    if 2 <= nb <= 7:
                        j = nb - 2
                        nc.gpsimd.tensor_add(colT[j], colT[j], ex)
                    elif nb == 1 and mb < 4:
                        nc.gpsimd.tensor_add(colT[6], colT[6], ex)
                    elif nb == 8:
                        nc.gpsimd.tensor_add(colT[7], colT[7], ex)
                    elif nb == 9:
                        nc.gpsimd.tensor_add(colT[8], colT[8], ex)

            # ---- colsum partition-reduce via ones-matmuls into one psum tile ----
            csps = psum_cs.tile([P, 36], f32, tag="csps")
            for j in range(9):
                ctb = work.tile([P, COLG], bf16, tag="ctb")
                nc.vector.tensor_copy(ctb, colT[j])
                for q in range(COLG // P):
                    nc.tensor.matmul(
                        csps[:, 4 * j + q : 4 * j + q + 1],
                        lhsT=ctb[:, q * P : (q + 1) * P],
                        rhs=ones_col,
                        start=True,
                        stop=True,
                    )
            nc.scalar.copy(cs, csps)

            # ---- pack + AllGather ----
            rowsum = accp.tile([P, MB], f32, tag="rowsum")
            nc.vector.tensor_reduce(
                rowsum, accs, axis=mybir.AxisListType.X, op=ALU.add
            )
            # rows [512:1024) of the own block also need Q01's colsums (d0
            # bottom-left quadrant contribution), cs cols 24..27
            nc.vector.tensor_add(rowsum[:, 4:8], rowsum[:, 4:8], cs[:, 24:28])
            pospk = accp.tile([P, 1], f32, tag="pospk")
            nc.vector.tensor_reduce(
                pospk, posq, axis=mybir.AxisListType.X, op=ALU.add
            )
            d1 = nc.sync.dma_start(
                out=pkt_dram[PK_ROWSUM : PK_ROWSUM + 1024].rearrange(
                    "(m p) -> p m", p=P
                ),
                in_=rowsum,
            )
            d2 = nc.sync.dma_start(
                out=pkt_dram[PK_CS1 : PK_CS1 + 3072].rearrange("(c p) -> p c", p=P),
                in_=cs[:, 0:24],
            )
            d3 = nc.sync.dma_start(
                out=pkt_dram[PK_CS4A : PK_CS4A + 1024].rearrange("(c p) -> p c", p=P),
                in_=cs[:, 28:36],
            )
            d4 = nc.sync.dma_start(
                out=pkt_dram[PK_POS : PK_POS + P].rearrange("(a p) -> p a", p=P),
                in_=pospk,
            )
            cc = nc.gpsimd.collective_compute(
                "AllGather",
                mybir.AluOpType.bypass,
                ins=[pkt_dram.ap()],
                outs=[gathered.ap()],
                replica_groups=[list(range(N_CORES))],
            )
            for d in (d1, d2, d3, d4):
                _add_dep_helper(cc.ins, d.ins, reason="cc after pkt")

            # ---- reassemble global totals; ln; global reduce ----
            # tot[p, b, m] = total exp-sum for global row 1024 b + 128 m + p
            Rt = accp.tile([P, N_CORES, MB], f32, tag="Rt")
            tot = accp.tile([P, N_CORES, MB], f32, tag="tot")
            Cd = {
                d: accp.tile([P, N_CORES, MB], f32, tag=f"Cd{d}", name=f"Cd{d}")
                for d in (1, 2, 3, 4)
            }
            posv = accp.tile([P, 4], f32, tag="posv")
            for b in range(N_CORES):
                dr = nc.sync.dma_start(
                    out=Rt[:, b, :],
                    in_=gathered[b, PK_ROWSUM : PK_ROWSUM + 1024].rearrange(
                        "(m p) -> p m", p=P
                    ),
                )
                _add_dep_helper(dr.ins, cc.ins, reason="read gathered after cc")
                for d in (1, 2, 3):
                    s = (b - d) % N_CORES
                    dc = nc.sync.dma_start(
                        out=Cd[d][:, b, :],
                        in_=gathered[s, 1024 * d : 1024 * (d + 1)].rearrange(
                            "(m p) -> p m", p=P
                        ),
                    )
                    _add_dep_helper(dc.ins, cc.ins, reason="read gathered after cc")
                # d4: from rank rb = (b+4)%8. Its cs_d4A covers block-b rows
                # [0:512) iff rb < 4, else rows [512:1024); cs_d4B the rest.
                rb = (b + 4) % N_CORES
                if rb < 4:
                    segs = [(PK_CS4A, 0), (PK_CS4B, 4)]
                else:
                    segs = [(PK_CS4A, 4), (PK_CS4B, 0)]
                for base, mhalf in segs:
                    dc4 = nc.sync.dma_start(
                        out=Cd[4][:, b, mhalf : mhalf + 4],
                        in_=gathered[rb, base : base + 512].rearrange(
                            "(m p) -> p m", p=P
                        ),
                    )
                    _add_dep_helper(dc4.ins, cc.ins, reason="read gathered after cc")
            for r in range(4):
                dp = nc.sync.dma_start(
                    out=posv[:, r : r + 1],
                    in_=gathered[r, PK_POS : PK_POS + P].rearrange(
                        "(a p) -> p a", p=P
                    ),
                )
                _add_dep_helper(dp.ins, cc.ins, reason="read gathered after cc")

            nc.vector.tensor_copy(tot, Rt)
            for d in (1, 2, 3, 4):
                nc.vector.tensor_add(tot, tot, Cd[d])

            lnt = accp.tile([P, N_CORES, MB], f32, tag="lnt")
            nc.scalar.activation(lnt, tot, AF.Ln)
            gsum = accp.tile([P, 1], f32, tag="gsum")
            nc.vector.tensor_reduce(
                gsum, lnt, axis=mybir.AxisListType.XY, op=ALU.add
            )
            # pos: ranks 0..3 hold the unique d4-diagonal sums; each entry
            # serves two global rows -> factor 2
            psum4 = accp.tile([P, 1], f32, tag="psum4")
            nc.vector.tensor_reduce(
                psum4, posv, axis=mybir.AxisListType.X, op=ALU.add
            )
            part = accp.tile([P, 1], f32, tag="part")
            nc.vector.tensor_scalar_mul(
                part, psum4, -2.0 * INV_T / (SCALE * SCALE) / N_CORES
            )
            gpart = accp.tile([P, 1], f32, tag="gpart")
            nc.vector.tensor_scalar_mul(gpart, gsum, 1.0 / N_CORES)
            nc.vector.tensor_add(part, part, gpart)
            nc.sync.dma_start(out=out_dram[:, :], in_=part)

    nc.finalize()
    return nc


def _get_nc():
    if "nc" not in _CACHE:
        _CACHE["nc"] = _build_nc()
    return _CACHE["nc"]


def _run(z, trace=False):
    from concourse.bass_utils import run_bass_kernel_spmd

    z = np.ascontiguousarray(np.asarray(z, dtype=np.float32))
    assert z.shape == (N, D), z.shape
    nc = _get_nc()
    in_maps = make_in_maps(z)
    res = run_bass_kernel_spmd(
        nc, in_maps, core_ids=list(range(N_CORES)), trace=False
    )
    total = np.float64(0.0)
    for r in res.results:
        total += r["out"].astype(np.float64).sum()
    loss = np.float32(total / N)
    return loss, res


def kernel(z):
    loss, _ = _run(z, trace=False)
    return np.array(loss, dtype=np.float32)


# revision 9
# speedup vs baseline: 1.0082x; 1.0082x over previous
"""Contrastive-loss Trainium2 kernel v2: symmetry-maximized fp8 GEMM + AllGather.

Over the v1 baseline:
  - DoubleRowSwInterleave perf mode (contiguous weight loads, ~1.6x faster MMs)
  - d=4 block pair computed once globally: each core does two quadrant pieces
    (pieceA = rows[0:512) x d4cols[0:512), pieceB = rows[512:1024) x
    d4cols[512:1024) in the LOCAL frame; the host permutes the d4 column rows
    for cores >= 4 so the four unique d4 blocks are tiled exactly once).
  - d=0 diagonal block: only quadrants Q00 (nb0 x mb0-3), Q01 (nb1 x mb0-3),
    Q11 (nb1 x mb4-7) are computed; rows[512:1024) get Q01's column sums
    locally (skips Q10: 4 psum tiles).
  - pos comes from the d4 piece diagonals: only ranks 0..3 hold real pos
    entries (each serving two global rows); assembly uses 2x their sums.

Per-core GEMM: 68 psum tiles [128,512] x 4 MMs (K=256 DoubleRow) = 272 MMs.

Packet (f32): [rowsum 1024 | cs_d1 1024 | cs_d2 1024 | cs_d3 1024 |
               cs_d4A 512 | cs_d4B 512 | pos 128] = 5248.
Row totals for block b rows: rowsum(rank b) + cs_d{1,2,3}(rank b-d)
  + cs_d4A/B(rank b+4; A covers rows[0:512) iff (b+4)%8 < 4, else swapped).
"""

import os
from contextlib import ExitStack

import numpy as np

N = 8192
D = 1024
N_CORES = 8
ROWS_PER_CORE = N // N_CORES  # 1024
P = 128
TEMPERATURE = 0.07
INV_T = 1.0 / TEMPERATURE
MASK_VAL = -65504.0
SCALE = 16.0  # pre-fp8 scale; psum holds SCALE^2 * cos

NROWS = 1024  # only own rows are loaded; column blocks arrive via AllGather
N_ROW_TILES = NROWS // P  # 8
MB = ROWS_PER_CORE // P  # 8
KT = D // P  # 8
COLG = 512
NB = 10  # column groups: 0,1 own; 2..7 d1..d3; 8,9 d4 pieces
MB_OF_NB = {
    0: range(0, 4),  # Q00
    1: range(0, 8),  # Q01 (mb 0-3) + Q11 (mb 4-7)
    8: range(0, 4),  # d4 piece A
    9: range(4, 8),  # d4 piece B
}
for _nb in range(2, 8):
    MB_OF_NB[_nb] = range(0, 8)

# packet field offsets (floats)
PK_ROWSUM = 0
PK_CS1 = 1024
PK_CS2 = 2048
PK_CS3 = 3072
PK_CS4A = 4096
PK_CS4B = 4608
PK_POS = 5120
PKT = 5248

_CACHE = {}


def make_in_maps(z):
    z = np.ascontiguousarray(np.asarray(z, dtype=np.float32))
    return [
        {"z": np.ascontiguousarray(z[ROWS_PER_CORE * c : ROWS_PER_CORE * (c + 1)])}
        for c in range(N_CORES)
    ]


def _build_nc(repeat=1):
    import concourse.mybir as mybir
    import concourse.tile as tile
    from concourse import bacc
    from concourse.masks import make_identity
    from concourse.bass import _add_dep_helper

    f32 = mybir.dt.float32
    bf16 = mybir.dt.bfloat16
    fp8 = mybir.dt.float8e4
    AF = mybir.ActivationFunctionType
    ALU = mybir.AluOpType

    nc = bacc.Bacc("TRN2")
    z_in = nc.dram_tensor("z", [NROWS, D], f32, kind="ExternalInput")
    out_dram = nc.dram_tensor("out", [P, 1], f32, kind="ExternalOutput")
    pkt_dram = nc.dram_tensor("pkt", [PKT], f32)
    gathered = nc.dram_tensor("gathered", [N_CORES, PKT], f32, addr_space="Shared")
    # fp8 znt exchange: each rank publishes its 2 own groups (1MB)
    zpkt = nc.dram_tensor("zpkt", [2, P, KT // 2, 2 * COLG], fp8)
    gathered_z = nc.dram_tensor(
        "gathered_z", [2 * N_CORES, P, KT // 2, 2 * COLG], fp8, addr_space="Shared"
    )

    ctx = ExitStack()
    with ctx:
        tc = ctx.enter_context(tile.TileContext(nc))
        consts = ctx.enter_context(tc.tile_pool(name="consts", bufs=1))
        znt_pool = ctx.enter_context(tc.tile_pool(name="znt", bufs=1))
        work = ctx.enter_context(tc.tile_pool(name="work", bufs=3))
        zin = ctx.enter_context(tc.tile_pool(name="zin", bufs=4))
        small = ctx.enter_context(tc.tile_pool(name="small", bufs=4))
        accp = ctx.enter_context(tc.tile_pool(name="accp", bufs=1))
        colp = ctx.enter_context(tc.tile_pool(name="colp", bufs=1))
        psum_t = ctx.enter_context(tc.tile_pool(name="psum_t", bufs=1, space="PSUM"))
        psum_mm = ctx.enter_context(tc.tile_pool(name="psum_mm", bufs=5, space="PSUM"))
        psum_cs = ctx.enter_context(tc.tile_pool(name="psum_cs", bufs=1, space="PSUM"))

        ident_f32 = consts.tile([P, P], f32, tag="ident_f32")
        make_identity(nc, ident_f32)
        ident_bf16 = consts.tile([P, P], bf16, tag="ident_bf16")
        make_identity(nc, ident_bf16)
        # anti-identity: permutation that reverses columns in PE transpose
        anti_bf16 = consts.tile([P, P], bf16, tag="anti_bf16")
        nc.gpsimd.memset(anti_bf16, 0.0)
        nc.gpsimd.affine_select(
            out=anti_bf16,
            in_=anti_bf16,
            compare_op=mybir.AluOpType.not_equal,
            fill=1.0,
            base=-(P - 1),
            # on where (x + y - 127) == 0
            pattern=[[1, P]],
            channel_multiplier=1,
        )
        negtile = consts.tile([P, P], f32, tag="negtile")
        nc.vector.memset(negtile, MASK_VAL * SCALE * SCALE)
        ident_u8 = consts.tile([P, P], mybir.dt.uint8, tag="ident_u8")
        nc.vector.tensor_copy(ident_u8, ident_f32)
        ones_col = consts.tile([P, 1], bf16, tag="ones_col")
        nc.vector.memset(ones_col, 1.0)

        # interleaved layout for DoubleRowSwInterleave: znt[g] is [P, 4, 1024]
        # fp8 with flat index (per kk2 group) s = 2*c + i holding
        # znT[k = 128*(2*kk2 + i) + p, 512*g + c]
        znt = [
            znt_pool.tile([P, KT // 2, 2 * COLG], fp8, tag=f"znt{g}", name=f"znt{g}")
            for g in range(NB)
        ]
        # weights copies of the own-row groups (lg 0,1): within each 128-col
        # chunk the columns are REVERSED (via anti-identity transpose), giving
        # the [A_{m+127} B_{m+127} ... A_m B_m] flat order SwInterleave wants
        znt_w = [
            znt_pool.tile([P, KT // 2, 2 * COLG], fp8, tag=f"zntw{g}", name=f"zntw{g}")
            for g in range(2)
        ]

        accs = accp.tile([P, MB, NB], f32, tag="accs")
        posq = accp.tile([P, MB], f32, tag="posq")
        # column-sum accumulators:
        #  colT[0..5] <- nb 2..7 (all mb)     -> cs_d1..cs_d3
        #  colT[6]    <- nb 1, mb 0..3        -> Q01 colsums (local use)
        #  colT[7]    <- nb 8, mb 0..3        -> cs_d4A
        #  colT[8]    <- nb 9, mb 4..7        -> cs_d4B
        colT = [
            colp.tile([P, COLG], f32, tag=f"colT{j}", name=f"colT{j}")
            for j in range(9)
        ]
        cs = colp.tile([P, 36], f32, tag="cs")  # 9 groups x 4 chunks

        for _rep in range(repeat):
            # ---- phase 1: normalize + transpose (local rows [0:5120)) ----
            for t in range(N_ROW_TILES):
                zt = zin.tile([P, 2, D // 2], f32, tag="zt")
                nc.sync.dma_start(
                    out=zt,
                    in_=z_in[t * P : (t + 1) * P, :].rearrange(
                        "p (a b) -> p a b", a=2
                    ),
                )
                sq = work.tile([P, 2, D // 2], bf16, tag="sq")
                s2 = small.tile([P, 1], f32, tag="s2")
                nc.scalar.activation(sq, zt, AF.Square, accum_out=s2)
                nrm = small.tile([P, 1], f32, tag="nrm")
                # nrm = ||z_row|| / SCALE  (norms ~32 >> EPS, clamp not needed)
                nc.scalar.activation(nrm, s2, AF.Sqrt, scale=1.0 / (SCALE * SCALE))
                rinv = small.tile([P, 1], f32, tag="rinv")
                nc.vector.reciprocal(rinv, nrm)

                zn_row = work.tile([P, D], bf16, tag="zn_row")
                nc.vector.tensor_scalar_mul(
                    zn_row.rearrange("p (a b) -> p a b", a=2), zt, rinv
                )

                ptr = psum_t.tile([P, KT * P], bf16, tag="ptr")
                for kk in range(KT):
                    nc.tensor.transpose(
                        ptr[:, kk * P : (kk + 1) * P],
                        zn_row[:, kk * P : (kk + 1) * P],
                        ident_bf16,
                    )
                g, col = t // 4, (t % 4) * P
                # interleave into znt: dst[p, kk2, c, i] = ptr[p, (2kk2+i)*128 + (c-col)]
                dstv = znt[g].rearrange("p k (c i) -> p k c i", i=2)
                srcv = ptr.rearrange("p (k2 j c) -> p k2 j c", k2=KT // 2, j=2)
                for i in range(2):
                    dst = dstv[:, :, col : col + P, i]
                    src = srcv[:, :, i, :]
                    if (t + i) % 2 == 0:
                        nc.scalar.copy(dst, src)
                    else:
                        nc.vector.tensor_copy(dst, src)
                if t < 8:
                    # own rows: also build the reversed weights copy
                    ptr2 = psum_t.tile([P, KT * P], bf16, tag="ptr2")
                    for kk in range(KT):
                        nc.tensor.transpose(
                            ptr2[:, kk * P : (kk + 1) * P],
                            zn_row[:, kk * P : (kk + 1) * P],
                            anti_bf16,
                        )
                    dstw = znt_w[g].rearrange("p k (c i) -> p k c i", i=2)
                    srcw = ptr2.rearrange("p (k2 j c) -> p k2 j c", k2=KT // 2, j=2)
                    for i in range(2):
                        dst = dstw[:, :, col : col + P, i]
                        src = srcw[:, :, i, :]
                        if (t + i) % 2 == 0:
                            nc.vector.tensor_copy(dst, src)
                        else:
                            nc.scalar.copy(dst, src)

            # ---- publish own fp8 groups; gather everyone's ----
            import concourse.bass as bass_mod

            zw = [
                nc.sync.dma_start(out=zpkt[g], in_=znt[g]) for g in range(2)
            ]
            ccz = nc.gpsimd.collective_compute(
                "AllGather",
                mybir.AluOpType.bypass,
                ins=[zpkt.ap()],
                outs=[gathered_z.ap()],
                replica_groups=[list(range(N_CORES))],
            )
            for d in zw:
                _add_dep_helper(ccz.ins, d.ins, reason="ccz after zpkt write")
            pid = nc.sync.partition_id()
            # distance-d blocks -> znt[2d], znt[2d+1]; d4 half-blocks by group:
            # pieceA = group (pid//4) of rank (pid+4)%8, pieceB = the other
            for d in (1, 2, 3):
                for g in range(2):
                    idx = ((pid + d) % 8) * 2 + g
                    dz = nc.sync.dma_start(
                        out=znt[2 * d + g],
                        in_=gathered_z[bass_mod.ds(idx, 1)],
                    )
                    _add_dep_helper(dz.ins, ccz.ins, reason="read z after ccz")
            gA = pid // 4
            idxA = ((pid + 4) % 8) * 2 + gA
            idxB = ((pid + 4) % 8) * 2 + (1 - gA)
            for slot, idx in ((8, idxA), (9, idxB)):
                dz = nc.sync.dma_start(
                    out=znt[slot], in_=gathered_z[bass_mod.ds(idx, 1)]
                )
                _add_dep_helper(dz.ins, ccz.ins, reason="read z after ccz")

            for j in range(9):
                nc.vector.memset(colT[j], 0.0)

            # ---- phase 2: GEMM + exp row-sums + colsum accumulation ----
            for nb in range(NB):
                for mb in MB_OF_NB[nb]:
                    ps = psum_mm.tile([P, COLG], f32, tag="ps")
                    lg, lcol = mb // 4, (mb % 4) * P
                    for kk2 in range(KT // 2):
                        w_ap = znt_w[lg][:, kk2, 2 * lcol : 2 * (lcol + P)].rearrange(
                            "p (m i) -> p m i", i=2
                        )
                        x_ap = znt[nb][:, kk2, :].rearrange("p (n i) -> p i n", i=2)
                        nc.tensor.matmul(
                            ps,
                            lhsT=w_ap,
                            rhs=x_ap,
                            perf_mode=mybir.MatmulPerfMode.DoubleRowSwInterleave,
                            start=(kk2 == 0),
                            stop=(kk2 == KT // 2 - 1),
                        )
                    # self-similarity masking on the d0 diagonal tiles
                    if nb == mb // 4:
                        off = (mb % 4) * P
                        nc.vector.copy_predicated(
                            out=ps[:, off : off + P], mask=ident_u8, data=negtile
                        )
                    # pos extraction from the d4 piece diagonals
                    if (nb == 8 and mb < 4) or (nb == 9 and mb >= 4):
                        off = (mb % 4) * P
                        pos_scr = work.tile([P, P], f32, tag="pos_scr")
                        nc.vector.tensor_mul(pos_scr, ps[:, off : off + P], ident_f32)
                        nc.vector.tensor_reduce(
                            posq[:, mb : mb + 1],
                            pos_scr,
                            axis=mybir.AxisListType.X,
                            op=ALU.add,
                        )
                    ex = work.tile([P, COLG], bf16, tag="ex")
                    nc.scalar.activation(
                        ex, ps, AF.Exp, scale=INV_T / (SCALE * SCALE),
                        accum_out=accs[:, mb, nb : nb + 1],
                    )
                    if 2 <= nb <= 7:
                        j = nb - 2
                        nc.gpsimd.tensor_add(colT[j], colT[j], ex)
                    elif nb == 1 and mb < 4:
                        nc.gpsimd.tensor_add(colT[6], colT[6], ex)
                    elif nb == 8:
                        nc.gpsimd.tensor_add(colT[7], colT[7], ex)
                    elif nb == 9:
                        nc.gpsimd.tensor_add(colT[8], colT[8], ex)

            # ---- colsum partition-reduce via ones-matmuls into one psum tile ----
            csps = psum_cs.tile([P, 36], f32, tag="csps")
            for j in range(9):
                ctb = work.tile([P, COLG], bf16, tag="ctb")
                nc.vector.tensor_copy(ctb, colT[j])
                for q in range(COLG // P):
                    nc.tensor.matmul(
                        csps[:, 4 * j + q : 4 * j + q + 1],
                        lhsT=ctb[:, q * P : (q + 1) * P],
                        rhs=ones_col,
                        start=True,
                        stop=True,
                    )
            nc.scalar.copy(cs, csps)

            # ---- pack + AllGather ----
            rowsum = accp.tile([P, MB], f32, tag="rowsum")
            nc.vector.tensor_reduce(
                rowsum, accs, axis=mybir.AxisListType.X, op=ALU.add
            )
            # rows [512:1024) of the own block also need Q01's colsums (d0
            # bottom-left quadrant contribution), cs cols 24..27
            nc.vector.tensor_add(rowsum[:, 4:8], rowsum[:, 4:8], cs[:, 24:28])
            pospk = accp.tile([P, 1], f32, tag="pospk")
            nc.vector.tensor_reduce(
                pospk, posq, axis=mybir.AxisListType.X, op=ALU.add
            )
            d1 = nc.sync.dma_start(
                out=pkt_dram[PK_ROWSUM : PK_ROWSUM + 1024].rearrange(
                    "(m p) -> p m", p=P
                ),
                in_=rowsum,
            )
            d2 = nc.sync.dma_start(
                out=pkt_dram[PK_CS1 : PK_CS1 + 3072].rearrange("(c p) -> p c", p=P),
                in_=cs[:, 0:24],
            )
            d3 = nc.sync.dma_start(
                out=pkt_dram[PK_CS4A : PK_CS4A + 1024].rearrange("(c p) -> p c", p=P),
                in_=cs[:, 28:36],
            )
            d4 = nc.sync.dma_start(
                out=pkt_dram[PK_POS : PK_POS + P].rearrange("(a p) -> p a", p=P),
                in_=pospk,
            )
            cc = nc.gpsimd.collective_compute(
                "AllGather",
                mybir.AluOpType.bypass,
                ins=[pkt_dram.ap()],
                outs=[gathered.ap()],
                replica_groups=[list(range(N_CORES))],
            )
            for d in (d1, d2, d3, d4):
                _add_dep_helper(cc.ins, d.ins, reason="cc after pkt")

            # ---- reassemble global totals; ln; global reduce ----
            # One contiguous DMA per rank: allF[p, b, f] = gathered[b, 128f + p]
            # f 0-7: rowsum(m); 8-15/16-23/24-31: cs_d1/2/3(m); 32-35: cs_d4A;
            # 36-39: cs_d4B; 40: pos
            NF = PKT // P  # 41
            allF = accp.tile([P, N_CORES, NF], f32, tag="allF")
            for b in range(N_CORES):
                dr = nc.sync.dma_start(
                    out=allF[:, b, :],
                    in_=gathered[b, :].rearrange("(f p) -> p f", p=P),
                )
                _add_dep_helper(dr.ins, cc.ins, reason="read gathered after cc")

            # tot[p, b, m] = total exp-sum for global row 1024 b + 128 m + p
            tot = accp.tile([P, N_CORES, MB], f32, tag="tot")
            nc.vector.tensor_copy(tot, allF[:, :, 0:MB])
            for d in (1, 2, 3):
                # block b gets cs_d from rank (b - d) % 8
                f0 = MB * d
                nc.vector.tensor_add(
                    tot[:, d:8, :], tot[:, d:8, :], allF[:, 0 : 8 - d, f0 : f0 + MB]
                )
                nc.vector.tensor_add(
                    tot[:, 0:d, :], tot[:, 0:d, :], allF[:, 8 - d : 8, f0 : f0 + MB]
                )
            # d4: block b <- rank rb=(b+4)%8; cs_d4A covers rows[0:512) iff
            # rb < 4 else rows[512:1024); cs_d4B the rest.
            for f0, b0, mhalf in ((32, 0, 4), (36, 0, 0), (32, 4, 0), (36, 4, 4)):
                rb0 = (b0 + 4) % N_CORES
                nc.vector.tensor_add(
                    tot[:, b0 : b0 + 4, mhalf : mhalf + 4],
                    tot[:, b0 : b0 + 4, mhalf : mhalf + 4],
                    allF[:, rb0 : rb0 + 4, f0 : f0 + 4],
                )

            lnt = accp.tile([P, N_CORES, MB], f32, tag="lnt")
            nc.scalar.activation(lnt, tot, AF.Ln)
            gsum = accp.tile([P, 1], f32, tag="gsum")
            nc.vector.tensor_reduce(
                gsum, lnt, axis=mybir.AxisListType.XY, op=ALU.add
            )
            # pos: ranks 0..3 hold the unique d4-diagonal sums; each entry
            # serves two global rows -> factor 2
            psum4 = accp.tile([P, 1], f32, tag="psum4")
            nc.vector.tensor_reduce(
                psum4, allF[:, 0:4, 40], axis=mybir.AxisListType.X, op=ALU.add
            )
            part = accp.tile([P, 1], f32, tag="part")
            nc.vector.tensor_scalar_mul(
                part, psum4, -2.0 * INV_T / (SCALE * SCALE) / N_CORES
            )
            gpart = accp.tile([P, 1], f32, tag="gpart")
            nc.vector.tensor_scalar_mul(gpart, gsum, 1.0 / N_CORES)
            nc.vector.tensor_add(part, part, gpart)
            nc.sync.dma_start(out=out_dram[:, :], in_=part)

    nc.finalize()
    return nc


def _get_nc():
    if "nc" not in _CACHE:
        _CACHE["nc"] = _build_nc()
    return _CACHE["nc"]


def _run(z, trace=False):
    from concourse.bass_utils import run_bass_kernel_spmd

    z = np.ascontiguousarray(np.asarray(z, dtype=np.float32))
    assert z.shape == (N, D), z.shape
    nc = _get_nc()
    in_maps = make_in_maps(z)
    res = run_bass_kernel_spmd(
        nc, in_maps, core_ids=list(range(N_CORES)), trace=False
    )
    total = np.float64(0.0)
    for r in res.results:
        total += r["out"].astype(np.float64).sum()
    loss = np.float32(total / N)
    return loss, res


def kernel(z):
    loss, _ = _run(z, trace=False)
    return np.array(loss, dtype=np.float32)
